# revision 3
# baseline (speedup 1.0000x reference)
"""MoE gating kernel for Trainium2 (Bass/Tile), data-parallel over 8 NeuronCores.

Computes: logits = x @ W_g.T ; top-2 values; softmax over the 2 values.
  p1 = sigmoid(v1 - v2), p2 = sigmoid(v2 - v1)  (v1 >= v2 the top-2 logits)

Sharding: tokens split 8 ways (2048 tokens/core), W_g replicated.
"""

import sys

sys.path.insert(0, "/opt/trn_rl_repo")

from contextlib import ExitStack

import numpy as np

import concourse.bass as bass
import concourse.bacc as bacc
import concourse.mybir as mybir
from concourse import masks
from concourse.tile import TileContext
from concourse.bass_utils import run_bass_kernel_spmd

TOKENS = 16384
DIM = 2048
E = 64  # num experts
NCORES = 8
TPC = TOKENS // NCORES  # tokens per core
P = 128  # partitions
KT = DIM // P  # 16 contraction tiles
NTT = TPC // P  # 16 token tiles per core

F32 = mybir.dt.float32


def _emit(tc: TileContext, ctx: ExitStack, x_ap, wg_ap, out_ap):
    nc = tc.nc

    singles = ctx.enter_context(tc.tile_pool(name="singles", bufs=1))
    xpool = ctx.enter_context(tc.tile_pool(name="xpool", bufs=2))
    xtpool = ctx.enter_context(tc.tile_pool(name="xtpool", bufs=2))
    spool = ctx.enter_context(tc.tile_pool(name="spool", bufs=4))
    opool = ctx.enter_context(tc.tile_pool(name="opool", bufs=4))
    psum_t = ctx.enter_context(tc.tile_pool(name="psum_t", bufs=4, space="PSUM"))
    psum_l = ctx.enter_context(tc.tile_pool(name="psum_l", bufs=4, space="PSUM"))

    ident = singles.tile([P, P], F32)
    masks.make_identity(nc, ident[:])

    # --- one-time: load W_g [64, 2048] and transpose to wgT [128, 16*64] ---
    wg_sb = singles.tile([E, DIM], F32)
    nc.sync.dma_start(out=wg_sb[:], in_=wg_ap)
    wgT = singles.tile([P, KT * E], F32)  # [d-part, k-tile * expert]
    for k in range(KT):
        pt = psum_t.tile([P, E], F32)
        nc.tensor.matmul(
            pt[:],
            wg_sb[:, k * P : (k + 1) * P],  # [64, 128]
            ident[:E, :E],
            is_transpose=True,
        )
        nc.vector.tensor_copy(wgT[:, k * E : (k + 1) * E], pt[:])

    # --- per token-tile pipeline ---
    for ti in range(NTT):
        x_tile = xpool.tile([P, DIM], F32)
        nc.sync.dma_start(out=x_tile[:], in_=x_ap[ti * P : (ti + 1) * P, :])

        # transpose x tile: [128 t, 2048 d] -> xt [128 d, 16k * 128 t]
        xt = xtpool.tile([P, KT * P], F32)
        for k in range(KT):
            pt = psum_t.tile([P, P], F32)
            nc.tensor.matmul(
                pt[:],
                x_tile[:, k * P : (k + 1) * P],
                ident[:],
                is_transpose=True,
            )
            nc.vector.tensor_copy(xt[:, k * P : (k + 1) * P], pt[:])

        # logits [128 t, 64 e] += xt_k.T @ wgT_k
        lp = psum_l.tile([P, E], F32)
        for k in range(KT):
            nc.tensor.matmul(
                lp[:],
                xt[:, k * P : (k + 1) * P],  # [128 d, 128 t] stationary
                wgT[:, k * E : (k + 1) * E],  # [128 d, 64 e] moving
                start=(k == 0),
                stop=(k == KT - 1),
            )

        # top-2 + softmax
        max8 = spool.tile([P, 8], F32)
        nc.vector.max(out=max8[:], in_=lp[:])
        d21 = spool.tile([P, 1], F32)
        nc.vector.tensor_sub(d21[:], max8[:, 1:2], max8[:, 0:1])  # v2 - v1
        ot = opool.tile([P, 2], F32)
        nc.scalar.activation(
            ot[:, 0:1], d21[:], mybir.ActivationFunctionType.Sigmoid, scale=-1.0
        )
        nc.scalar.activation(
            ot[:, 1:2], d21[:], mybir.ActivationFunctionType.Sigmoid, scale=1.0
        )
        nc.sync.dma_start(out=out_ap[ti * P : (ti + 1) * P, :], in_=ot[:])


_NC_CACHE = {}


def _build():
    if "nc" in _NC_CACHE:
        return _NC_CACHE["nc"]
    nc = bacc.Bacc(trn_type="TRN2")
    x = nc.dram_tensor("x", [TPC, DIM], F32, kind="ExternalInput")
    wg = nc.dram_tensor("w_g", [E, DIM], F32, kind="ExternalInput")
    out = nc.dram_tensor("out", [TPC, 2], F32, kind="ExternalOutput")
    with TileContext(nc) as tc, ExitStack() as ctx:
        _emit(tc, ctx, x.ap(), wg.ap(), out.ap())
    if not nc.is_finalized():
        nc.finalize()
    _NC_CACHE["nc"] = nc
    return nc


def _run(x, W_g, trace=False):
    nc = _build()
    x = np.ascontiguousarray(np.asarray(x, dtype=np.float32))
    W_g = np.ascontiguousarray(np.asarray(W_g, dtype=np.float32))
    in_maps = [
        {"x": np.ascontiguousarray(x[c * TPC : (c + 1) * TPC]), "w_g": W_g}
        for c in range(NCORES)
    ]
    res = run_bass_kernel_spmd(nc, in_maps, core_ids=list(range(NCORES)), trace=trace)
    out = np.concatenate([r["out"] for r in res.results], axis=0)
    return out, res


def kernel(x, W_g):
    out, _ = _run(x, W_g, trace=False)
    return out


def kernel_profiled(x, W_g):
    out, res = _run(x, W_g, trace=True)
    return out, res


# revision 10
# speedup vs baseline: 1.4514x; 1.4514x over previous
"""MoE gating kernel for Trainium2 (Bass/Tile), data-parallel over 8 NeuronCores.

Computes: logits = x @ W_g.T ; top-2 values; softmax over the 2 values.
  p1 = sigmoid(v1 - v2), p2 = sigmoid(v2 - v1)  (v1 >= v2 the top-2 logits)

Sharding: tokens split 8 ways (2048 tokens/core), W_g replicated.

Per-core structure (per 512-token group):
  - DMA 4 x-tiles [128, 2048]
  - PE-transpose x into xT [128 d, 16k * 512 t] via one-bank PSUM staging
  - 16 accumulating matmuls (float32r, N=512): logitsT [64 e, 512 t]
  - drain + PE-transpose back to [128 t, 64 e], DVE Max8 top-2, ACT sigmoid
"""

import sys

sys.path.insert(0, "/opt/trn_rl_repo")

from contextlib import ExitStack

import numpy as np

import concourse.bass as bass
import concourse.bacc as bacc
import concourse.mybir as mybir
from concourse import masks
from concourse.tile import TileContext
from concourse.bass_utils import run_bass_kernel_spmd

TOKENS = 16384
DIM = 2048
E = 64  # num experts
NCORES = 8
TPC = TOKENS // NCORES  # tokens per core
P = 128
KT = DIM // P  # 16 contraction tiles
G = 512  # token group (moving-dim of the big matmul)
NG = TPC // G  # 4 groups per core
TB = G // P  # 4 token blocks per group

F32 = mybir.dt.float32
F32R = mybir.dt.float32r

# which k-drains go to the scalar engine (ACT) instead of DVE (load balance)
ACT_DRAIN_EVERY = 4  # k % 4 == 3 -> ACT


def _emit(tc: TileContext, ctx: ExitStack, x_ap, wg_ap, out_ap, mm_f32r=True):
    nc = tc.nc

    singles = ctx.enter_context(tc.tile_pool(name="singles", bufs=1))
    xpool = ctx.enter_context(tc.tile_pool(name="xpool", bufs=2))
    xtpool = ctx.enter_context(tc.tile_pool(name="xtpool", bufs=2))
    ltpool = ctx.enter_context(tc.tile_pool(name="ltpool", bufs=2))
    spool = ctx.enter_context(tc.tile_pool(name="spool", bufs=4))
    opool = ctx.enter_context(tc.tile_pool(name="opool", bufs=4))
    psum_t = ctx.enter_context(tc.tile_pool(name="psum_t", bufs=3, space="PSUM"))
    psum_l = ctx.enter_context(tc.tile_pool(name="psum_l", bufs=2, space="PSUM"))
    psum_f = ctx.enter_context(tc.tile_pool(name="psum_f", bufs=3, space="PSUM"))

    ident = singles.tile([P, P], F32)
    masks.make_identity(nc, ident[:])

    # --- one-time: load W_g [64, 2048], transpose to wgT [128 d, 16k * 64 e] ---
    mmdt = F32R if mm_f32r else F32

    wg_sb = singles.tile([E, DIM], F32)
    nc.sync.dma_start(out=wg_sb[:], in_=wg_ap)
    wgT = singles.tile([P, KT * E], mmdt)
    for k in range(KT):
        pt = psum_f.tile([P, E], F32, tag="fin_ps")
        nc.tensor.matmul(
            pt[:],
            wg_sb[:, k * P : (k + 1) * P],
            ident[:E, :E],
            is_transpose=True,
        )
        nc.vector.tensor_copy(wgT[:, k * E : (k + 1) * E], pt[:])

    for g in range(NG):
        # load this group's 4 token blocks
        xtiles = []
        for tb in range(TB):
            xt_in = xpool.tile([P, DIM], F32, tag=f"x{tb}")
            r0 = g * G + tb * P
            nc.sync.dma_start(out=xt_in[:], in_=x_ap[r0 : r0 + P, :])
            xtiles.append(xt_in)

        # transpose into xT [128 d, k * 512 t]
        xt = xtpool.tile([P, KT * G], mmdt)
        for k in range(KT):
            pt = psum_t.tile([P, G], F32)
            for tb in range(TB):
                nc.tensor.matmul(
                    pt[:, tb * P : (tb + 1) * P],
                    xtiles[tb][:, k * P : (k + 1) * P],
                    ident[:],
                    is_transpose=True,
                )
            dst = xt[:, k * G : (k + 1) * G]
            if k % ACT_DRAIN_EVERY == ACT_DRAIN_EVERY - 1:
                nc.scalar.copy(dst, pt[:])
            else:
                nc.vector.tensor_copy(dst, pt[:])

        # logitsT [64 e, 512 t] = sum_k wgT_k.T @ xT_k
        lp = psum_l.tile([E, G], F32)
        for k in range(KT):
            nc.tensor.matmul(
                lp[:],
                wgT[:, k * E : (k + 1) * E],
                xt[:, k * G : (k + 1) * G],
                start=(k == 0),
                stop=(k == KT - 1),
            )

        # back to token-major + top-2 + softmax
        lt = ltpool.tile([E, G], F32)
        nc.vector.tensor_copy(lt[:], lp[:])
        for tb in range(TB):
            fp = psum_f.tile([P, E], F32, tag="fin_ps")
            nc.tensor.matmul(
                fp[:],
                lt[:, tb * P : (tb + 1) * P],
                ident[:E, :E],
                is_transpose=True,
            )
            max8 = spool.tile([P, 8], F32)
            nc.vector.max(out=max8[:], in_=fp[:])
            d21 = spool.tile([P, 1], F32)
            nc.vector.tensor_sub(d21[:], max8[:, 1:2], max8[:, 0:1])  # v2 - v1
            ot = opool.tile([P, 2], F32)
            nc.scalar.activation(
                ot[:, 0:1], d21[:], mybir.ActivationFunctionType.Sigmoid, scale=-1.0
            )
            nc.scalar.activation(
                ot[:, 1:2], d21[:], mybir.ActivationFunctionType.Sigmoid, scale=1.0
            )
            r0 = g * G + tb * P
            nc.sync.dma_start(out=out_ap[r0 : r0 + P, :], in_=ot[:])


_NC_CACHE = {}


def _build(mm_f32r=True):
    key = ("nc", mm_f32r)
    if key in _NC_CACHE:
        return _NC_CACHE[key]
    nc = bacc.Bacc(trn_type="TRN2")
    x = nc.dram_tensor("x", [TPC, DIM], F32, kind="ExternalInput")
    wg = nc.dram_tensor("w_g", [E, DIM], F32, kind="ExternalInput")
    out = nc.dram_tensor("out", [TPC, 2], F32, kind="ExternalOutput")
    with TileContext(nc) as tc, ExitStack() as ctx:
        _emit(tc, ctx, x.ap(), wg.ap(), out.ap(), mm_f32r=mm_f32r)
    if not nc.is_finalized():
        nc.finalize()
    _NC_CACHE[key] = nc
    return nc


def _run(x, W_g, trace=False, mm_f32r=True):
    nc = _build(mm_f32r=mm_f32r)
    x = np.ascontiguousarray(np.asarray(x, dtype=np.float32))
    W_g = np.ascontiguousarray(np.asarray(W_g, dtype=np.float32))
    in_maps = [
        {"x": np.ascontiguousarray(x[c * TPC : (c + 1) * TPC]), "w_g": W_g}
        for c in range(NCORES)
    ]
    res = run_bass_kernel_spmd(nc, in_maps, core_ids=list(range(NCORES)), trace=trace)
    out = np.concatenate([r["out"] for r in res.results], axis=0)
    return out, res


def kernel(x, W_g):
    out, _ = _run(x, W_g, trace=False)
    return out


def kernel_profiled(x, W_g, mm_f32r=True):
    out, res = _run(x, W_g, trace=True, mm_f32r=mm_f32r)
    return out, res


# revision 15
# speedup vs baseline: 1.4716x; 1.0139x over previous
"""MoE gating kernel for Trainium2 (Bass/Tile), data-parallel over 8 NeuronCores.

Computes: logits = x @ W_g.T ; top-2 values; softmax over the 2 values.
  p1 = sigmoid(v1 - v2), p2 = sigmoid(v2 - v1)  (v1 >= v2 the top-2 logits)

Sharding: tokens split 8 ways (2048 tokens/core), W_g replicated.

Per-core structure (per 512-token group):
  - DMA 4 x-tiles [128, 2048]
  - PE-transpose x into xT [128 d, 16k * 512 t] via one-bank PSUM staging
  - 16 accumulating matmuls (float32r, N=512): logitsT [64 e, 512 t]
  - drain + PE-transpose back to [128 t, 64 e], DVE Max8 top-2, ACT sigmoid
"""

import sys

sys.path.insert(0, "/opt/trn_rl_repo")

from contextlib import ExitStack

import numpy as np

import concourse.bass as bass
import concourse.bacc as bacc
import concourse.mybir as mybir
from concourse import masks
from concourse.tile import TileContext
from concourse.bass_utils import run_bass_kernel_spmd

TOKENS = 16384
DIM = 2048
E = 64  # num experts
NCORES = 8
TPC = TOKENS // NCORES  # tokens per core
P = 128
KT = DIM // P  # 16 contraction tiles
G = 512  # token group (moving-dim of the big matmul)
NG = TPC // G  # 4 groups per core
TB = G // P  # 4 token blocks per group

F32 = mybir.dt.float32
F32R = mybir.dt.float32r

# which k-drains go to the scalar engine (ACT) instead of DVE (load balance)
ACT_DRAIN_EVERY = 4  # k % 4 == 3 -> ACT


def _emit(tc: TileContext, ctx: ExitStack, x_ap, wg_ap, out_ap, mm_f32r=True):
    nc = tc.nc

    singles = ctx.enter_context(tc.tile_pool(name="singles", bufs=1))
    xpool = ctx.enter_context(tc.tile_pool(name="xpool", bufs=3))
    xtpool = ctx.enter_context(tc.tile_pool(name="xtpool", bufs=2))
    ltpool = ctx.enter_context(tc.tile_pool(name="ltpool", bufs=2))
    spool = ctx.enter_context(tc.tile_pool(name="spool", bufs=4))
    opool = ctx.enter_context(tc.tile_pool(name="opool", bufs=4))
    psum_t = ctx.enter_context(tc.tile_pool(name="psum_t", bufs=3, space="PSUM"))
    psum_l = ctx.enter_context(tc.tile_pool(name="psum_l", bufs=2, space="PSUM"))
    psum_f = ctx.enter_context(tc.tile_pool(name="psum_f", bufs=3, space="PSUM"))

    ident = singles.tile([P, P], F32)
    masks.make_identity(nc, ident[:])

    # --- one-time: load W_g [64, 2048], transpose to wgT [128 d, 16k * 64 e] ---
    mmdt = F32R if mm_f32r else F32

    wg_sb = singles.tile([E, DIM], F32)
    nc.sync.dma_start(out=wg_sb[:], in_=wg_ap)
    wgT = singles.tile([P, KT * E], mmdt)
    for k in range(KT):
        pt = psum_f.tile([P, E], F32, tag="fin_ps")
        nc.tensor.matmul(
            pt[:],
            wg_sb[:, k * P : (k + 1) * P],
            ident[:E, :E],
            is_transpose=True,
        )
        nc.vector.tensor_copy(wgT[:, k * E : (k + 1) * E], pt[:])

    for g in range(NG):
        # load this group's 4 token blocks
        xtiles = []
        for tb in range(TB):
            xt_in = xpool.tile([P, DIM], F32, tag=f"x{tb}")
            r0 = g * G + tb * P
            nc.sync.dma_start(out=xt_in[:], in_=x_ap[r0 : r0 + P, :])
            xtiles.append(xt_in)

        # transpose into xT [128 d, k * 512 t]
        xt = xtpool.tile([P, KT * G], mmdt)
        for k in range(KT):
            pt = psum_t.tile([P, G], F32)
            for tb in range(TB):
                nc.tensor.matmul(
                    pt[:, tb * P : (tb + 1) * P],
                    xtiles[tb][:, k * P : (k + 1) * P],
                    ident[:],
                    is_transpose=True,
                )
            dst = xt[:, k * G : (k + 1) * G]
            if k % ACT_DRAIN_EVERY == ACT_DRAIN_EVERY - 1:
                nc.scalar.copy(dst, pt[:])
            else:
                nc.vector.tensor_copy(dst, pt[:])

        # logitsT [64 e, 512 t] = sum_k wgT_k.T @ xT_k
        lp = psum_l.tile([E, G], F32)
        for k in range(KT):
            nc.tensor.matmul(
                lp[:],
                wgT[:, k * E : (k + 1) * E],
                xt[:, k * G : (k + 1) * G],
                start=(k == 0),
                stop=(k == KT - 1),
            )

        # back to token-major + top-2 + softmax
        lt = ltpool.tile([E, G], F32)
        nc.vector.tensor_copy(lt[:], lp[:])
        for tb in range(TB):
            fp = psum_f.tile([P, E], F32, tag="fin_ps")
            nc.tensor.matmul(
                fp[:],
                lt[:, tb * P : (tb + 1) * P],
                ident[:E, :E],
                is_transpose=True,
            )
            max8 = spool.tile([P, 8], F32)
            nc.vector.max(out=max8[:], in_=fp[:])
            d21 = spool.tile([P, 1], F32)
            nc.gpsimd.tensor_sub(d21[:], max8[:, 1:2], max8[:, 0:1])  # v2 - v1
            ot = opool.tile([P, 2], F32)
            nc.scalar.activation(
                ot[:, 0:1], d21[:], mybir.ActivationFunctionType.Sigmoid, scale=-1.0
            )
            nc.scalar.activation(
                ot[:, 1:2], d21[:], mybir.ActivationFunctionType.Sigmoid, scale=1.0
            )
            r0 = g * G + tb * P
            nc.sync.dma_start(out=out_ap[r0 : r0 + P, :], in_=ot[:])


_NC_CACHE = {}


def _build(mm_f32r=True):
    key = ("nc", mm_f32r)
    if key in _NC_CACHE:
        return _NC_CACHE[key]
    nc = bacc.Bacc(trn_type="TRN2")
    x = nc.dram_tensor("x", [TPC, DIM], F32, kind="ExternalInput")
    wg = nc.dram_tensor("w_g", [E, DIM], F32, kind="ExternalInput")
    out = nc.dram_tensor("out", [TPC, 2], F32, kind="ExternalOutput")
    with TileContext(nc) as tc, ExitStack() as ctx:
        _emit(tc, ctx, x.ap(), wg.ap(), out.ap(), mm_f32r=mm_f32r)
    if not nc.is_finalized():
        nc.finalize()
    _NC_CACHE[key] = nc
    return nc


def _run(x, W_g, trace=False, mm_f32r=True):
    nc = _build(mm_f32r=mm_f32r)
    x = np.ascontiguousarray(np.asarray(x, dtype=np.float32))
    W_g = np.ascontiguousarray(np.asarray(W_g, dtype=np.float32))
    in_maps = [
        {"x": np.ascontiguousarray(x[c * TPC : (c + 1) * TPC]), "w_g": W_g}
        for c in range(NCORES)
    ]
    res = run_bass_kernel_spmd(nc, in_maps, core_ids=list(range(NCORES)), trace=trace)
    out = np.concatenate([r["out"] for r in res.results], axis=0)
    return out, res


def kernel(x, W_g):
    out, _ = _run(x, W_g, trace=False)
    return out


def kernel_profiled(x, W_g, mm_f32r=True):
    out, res = _run(x, W_g, trace=True, mm_f32r=mm_f32r)
    return out, res
